# revision 19
# baseline (speedup 1.0000x reference)
# Trainium2 Bass kernel for nn_BinLinearEval:
#   out[b, o] = (round(x @ W.T + bias) * sign >= 0) ? 1.0 : 0.0
#
# Math folding (exact because bias is integer-valued and sign in {-1,+1}):
#   out = 1  iff  sign*(dot + bias) >= -0.5
#       = 1  iff  dot' >= thr_o      where dot' = x @ (sign.T*W).T  (W' still
#         ternary, exact in fp16) and thr_o = -sign_o*bias_o - 0.5.
# The device computes dot' in two accumulated passes — an fp16 hi pass plus
# an fp8-e4m3 DoubleRow residual pass (x_lo*2^6 vs W'*2^-6, both exactly
# representable; DoubleRow contracts K=256 per matmul at ~1.75x the fp16
# rate) — giving near-fp32 accuracy (20/16.7M threshold flips) at ~60% of
# the 2xfp16 cost. Epilogue is a single per-partition is_ge threshold.
#
# Sharding: data-parallel over batch, 8192 rows per core. x is pre-transposed
# on the host to [feature, batch] layout so the contract dim lands on SBUF
# partitions; output is produced as [out, batch] per core and re-assembled /
# transposed on the host.

import os
from contextlib import ExitStack

import numpy as np
import ml_dtypes

BATCH, IN_F, OUT_F = 65536, 1024, 256
N_CORES = 8
B_CORE = BATCH // N_CORES  # 8192
P = 128
KC = IN_F // P             # 8 k-chunks
OC = OUT_F // P            # 2 out-channel chunks
BT = 512                   # matmul moving free dim
# Uniform small groups + deep buffering: DMA stays saturated and the PE
# never outruns the prefetch pipeline by more than the buffer depth.
GROUPS = [512] * (B_CORE // 512)
assert sum(GROUPS) == B_CORE
IO_BUFS = 6
WARM_MMS = 16

_CACHE = {}


def _build():
    """Build (and cache) the Bass module. Returns the compiled nc."""
    if "nc" in _CACHE:
        return _CACHE["nc"]

    import concourse.bacc as bacc
    import concourse.mybir as mybir
    import concourse.tile as tile

    nc = bacc.Bacc(
        "TRN2",
        target_bir_lowering=False,
        debug=False,
        num_devices=N_CORES,
    )

    f16 = mybir.dt.float16
    f32 = mybir.dt.float32
    bf16 = mybir.dt.bfloat16
    f8 = mybir.dt.float8e4

    # group-major layouts: one group's slab is contiguous per partition
    # (8 KB / 4 KB descriptors instead of 1 KB / 512 B strided rows)
    n_groups = len(GROUPS)
    xhi_d = nc.dram_tensor(
        "xhi", [P, n_groups, KC, GROUPS[0]], f16, kind="ExternalInput"
    ).ap()
    xlo_d = nc.dram_tensor(
        "xlo8", [P, n_groups, KC // 2, 2, GROUPS[0]], f8, kind="ExternalInput"
    ).ap()
    wt_d = nc.dram_tensor("wt", [P, KC, OUT_F], f16, kind="ExternalInput").ap()
    wlo_d = nc.dram_tensor(
        "wlo8", [P, KC // 2, 2, OUT_F], f8, kind="ExternalInput"
    ).ap()
    thr_d = nc.dram_tensor("thr", [P, OC], f32, kind="ExternalInput").ap()
    out_d = nc.dram_tensor("out", [OC, P, B_CORE], bf16, kind="ExternalOutput").ap()

    with tile.TileContext(nc) as tc, ExitStack() as ctx:
        const = ctx.enter_context(tc.tile_pool(name="const", bufs=1))
        io = ctx.enter_context(tc.tile_pool(name="io", bufs=IO_BUFS))
        outp = ctx.enter_context(tc.tile_pool(name="outp", bufs=4))
        psum = ctx.enter_context(tc.tile_pool(name="psum", bufs=4, space="PSUM"))

        # consts ride the ACT HWDGE ring so the SP ring can start streaming
        # the first x group immediately; first matmul waits on whichever
        # finishes later (~2.8us instead of ~4.9us serialized)
        wt_sb = const.tile([P, KC, OUT_F], f16)
        nc.scalar.dma_start(out=wt_sb, in_=wt_d)
        wlo_sb = const.tile([P, KC // 2, 2, OUT_F], f8)
        nc.scalar.dma_start(out=wlo_sb, in_=wlo_d)
        thr_sb = const.tile([P, OC], f32)
        nc.scalar.dma_start(out=thr_sb, in_=thr_d)

        # HAM pre-warm: ~16 N=512 matmuls on memset scratch fill the initial
        # input-DMA window (~7us) with PE activity, so the activity monitor
        # un-throttles the PE clock (1.2->2.4 GHz) before real work arrives.
        # Sized to finish just as the first group lands; N=512 keeps any
        # tile-release sem spacing hidden under the 216-427ns stream time.
        warm_in = const.tile([P, BT], f16)
        nc.vector.memset(warm_in, 0.0)
        warm_ps = ctx.enter_context(
            tc.tile_pool(name="warm_ps", bufs=2, space="PSUM")
        )
        for _ in range(WARM_MMS):
            wps = warm_ps.tile([P, BT], f32, name="wps")
            nc.tensor.matmul(
                wps, warm_in[:, :P], warm_in, start=True, stop=True
            )

        g0 = 0
        for g, group in enumerate(GROUPS):
            xhi_sb = io.tile([P, KC, max(GROUPS)], f16, name="xhi_sb")[
                :, :, :group
            ]
            xlo_sb = io.tile([P, KC // 2, 2, max(GROUPS)], f8, name="xlo_sb")[
                :, :, :, :group
            ]
            nc.sync.dma_start(out=xhi_sb, in_=xhi_d[:, g])
            nc.sync.dma_start(out=xlo_sb, in_=xlo_d[:, g])
            for bt in range(group // BT):
                b0 = bt * BT
                for oc in range(OC):
                    ps = psum.tile([P, BT], f32, name="ps")
                    # all-hi then all-lo: the first matmuls of the kernel
                    # only need the hi half of the first group in SBUF
                    for k in range(KC):
                        nc.tensor.matmul(
                            ps,
                            wt_sb[:, k, oc * P : (oc + 1) * P],
                            xhi_sb[:, k, b0 : b0 + BT],
                            start=(k == 0),
                            stop=False,
                        )
                    # lo pass: fp8 e4m3 DoubleRow, contracts 256 per matmul
                    for c in range(KC // 2):
                        nc.tensor.matmul(
                            ps,
                            wlo_sb[:, c, :, oc * P : (oc + 1) * P],
                            xlo_sb[:, c, :, b0 : b0 + BT],
                            start=False,
                            stop=(c == KC // 2 - 1),
                            perf_mode=mybir.MatmulPerfMode.DoubleRow,
                        )
                    ob = outp.tile([P, BT], bf16, name="ob")
                    nc.vector.tensor_scalar(
                        ob,
                        ps,
                        thr_sb[:, oc : oc + 1],
                        None,
                        mybir.AluOpType.is_ge,
                    )
                    # out-DMAs ride the ACT HWDGE ring so they never block
                    # the input-DMA FIFO on the SP ring
                    nc.scalar.dma_start(
                        out=out_d[oc, :, g0 + b0 : g0 + b0 + BT], in_=ob
                    )
            g0 += group

    nc.compile()
    _CACHE["nc"] = nc
    return nc


def _prep_inputs(x, weight, bias, sign):
    """Host-side prep: fold sign into weights, build thresholds, split x into
    fp16 hi/lo, transpose to [feature, batch] per-core tiles."""
    x = np.asarray(x, dtype=np.float32)
    weight = np.asarray(weight, dtype=np.float32)
    bias = np.asarray(bias, dtype=np.float32)
    sign = np.asarray(sign, dtype=np.float32).reshape(1, OUT_F)

    wp = sign.T * weight                      # [OUT_F, IN_F], ternary
    thr = (-sign[0] * bias - np.float32(0.5)).astype(np.float32)  # [OUT_F]

    wt = np.ascontiguousarray(
        wp.T.reshape(KC, P, OUT_F).transpose(1, 0, 2)
    ).astype(np.float16)                      # [P, KC, OUT_F]
    thr2 = np.ascontiguousarray(thr.reshape(OC, P).T)  # [P, OC]

    xhi = x.astype(np.float16)
    f8np = ml_dtypes.float8_e4m3fn
    xlo8 = ((x - xhi.astype(np.float32)) * np.float32(64.0)).astype(f8np)
    wlo8 = np.ascontiguousarray(
        (wp.T * np.float32(1.0 / 64.0))
        .reshape(KC // 2, 2, P, OUT_F)
        .transpose(2, 0, 1, 3)
    ).astype(f8np)                            # [P, KC//2, 2, OUT_F]

    n_groups = len(GROUPS)
    grp = GROUPS[0]
    in_maps = []
    for c in range(N_CORES):
        sl = slice(c * B_CORE, (c + 1) * B_CORE)
        hi = np.ascontiguousarray(
            xhi[sl].reshape(n_groups, grp, KC, P).transpose(3, 0, 2, 1)
        )                                      # [P, n_groups, KC, grp]
        lo = np.ascontiguousarray(
            xlo8[sl]
            .reshape(n_groups, grp, KC // 2, 2, P)
            .transpose(4, 0, 2, 3, 1)
        )                                      # [P, n_groups, KC//2, 2, grp]
        in_maps.append(
            {"xhi": hi, "xlo8": lo, "wt": wt, "wlo8": wlo8, "thr": thr2}
        )
    return in_maps


def _assemble(results):
    """[core][OC, P, B_CORE] bf16 -> [BATCH, OUT_F] fp32"""
    full = np.concatenate(
        [r["out"].reshape(OUT_F, B_CORE) for r in results], axis=1
    )  # [OUT_F, BATCH]
    return np.ascontiguousarray(full.T).astype(np.float32)


def run(x, weight, bias, sign, trace=False):
    """Run the kernel; returns (output, BassKernelResults)."""
    from concourse.bass_utils import run_bass_kernel_spmd

    if not trace:
        # The NTFF profile hook module may be absent in this image; make
        # sure a stray BASS_TRACE=1 can't route us into the trace path.
        os.environ["BASS_NEVER_TRACE"] = "1"
    else:
        os.environ.pop("BASS_NEVER_TRACE", None)

    nc = _build()
    in_maps = _prep_inputs(x, weight, bias, sign)
    res = run_bass_kernel_spmd(
        nc,
        in_maps,
        core_ids=list(range(N_CORES)),
        trace=trace,
    )
    return _assemble(res.results), res


def kernel(x, weight, bias, sign):
    out, _ = run(x, weight, bias, sign, trace=False)
    return out


# revision 21
# speedup vs baseline: 1.0254x; 1.0254x over previous
# Trainium2 Bass kernel for nn_BinLinearEval:
#   out[b, o] = (round(x @ W.T + bias) * sign >= 0) ? 1.0 : 0.0
#
# Math folding (exact because bias is integer-valued and sign in {-1,+1}):
#   out = 1  iff  sign*(dot + bias) >= -0.5
#       = 1  iff  dot' >= thr_o      where dot' = x @ (sign.T*W).T  (W' still
#         ternary, exact in fp16) and thr_o = -sign_o*bias_o - 0.5.
# The device computes dot' in two accumulated passes — an fp16 hi pass plus
# an fp8-e4m3 DoubleRow residual pass (x_lo*2^6 vs W'*2^-6, both exactly
# representable; DoubleRow contracts K=256 per matmul at ~1.75x the fp16
# rate) — giving near-fp32 accuracy (20/16.7M threshold flips) at ~60% of
# the 2xfp16 cost. Epilogue is a single per-partition is_ge threshold.
#
# Sharding: data-parallel over batch, 8192 rows per core. x is pre-transposed
# on the host to [feature, batch] layout so the contract dim lands on SBUF
# partitions; output is produced as [out, batch] per core and re-assembled /
# transposed on the host.

import os
from contextlib import ExitStack

import numpy as np
import ml_dtypes

BATCH, IN_F, OUT_F = 65536, 1024, 256
N_CORES = 8
B_CORE = BATCH // N_CORES  # 8192
P = 128
KC = IN_F // P             # 8 k-chunks
OC = OUT_F // P            # 2 out-channel chunks
BT = 512                   # matmul moving free dim
# Uniform small groups + deep buffering: DMA stays saturated and the PE
# never outruns the prefetch pipeline by more than the buffer depth.
GROUPS = [512] * (B_CORE // 512)
assert sum(GROUPS) == B_CORE
IO_BUFS = 6

_CACHE = {}


def _build():
    """Build (and cache) the Bass module. Returns the compiled nc."""
    if "nc" in _CACHE:
        return _CACHE["nc"]

    import concourse.bacc as bacc
    import concourse.mybir as mybir
    import concourse.tile as tile

    nc = bacc.Bacc(
        "TRN2",
        target_bir_lowering=False,
        debug=False,
        num_devices=N_CORES,
    )

    f16 = mybir.dt.float16
    f32 = mybir.dt.float32
    bf16 = mybir.dt.bfloat16
    f8 = mybir.dt.float8e4

    # group-major layouts: one group's slab is contiguous per partition
    # (8 KB / 4 KB descriptors instead of 1 KB / 512 B strided rows)
    n_groups = len(GROUPS)
    xhi_d = nc.dram_tensor(
        "xhi", [P, n_groups, KC, GROUPS[0]], f16, kind="ExternalInput"
    ).ap()
    xlo_d = nc.dram_tensor(
        "xlo8", [P, n_groups, KC // 2, 2, GROUPS[0]], f8, kind="ExternalInput"
    ).ap()
    wt_d = nc.dram_tensor("wt", [P, KC, OUT_F], f16, kind="ExternalInput").ap()
    wlo_d = nc.dram_tensor(
        "wlo8", [P, KC // 2, 2, OUT_F], f8, kind="ExternalInput"
    ).ap()
    thr_d = nc.dram_tensor("thr", [P, OC], f32, kind="ExternalInput").ap()
    out_d = nc.dram_tensor("out", [OC, P, B_CORE], bf16, kind="ExternalOutput").ap()

    with tile.TileContext(nc) as tc, ExitStack() as ctx:
        const = ctx.enter_context(tc.tile_pool(name="const", bufs=1))
        io = ctx.enter_context(tc.tile_pool(name="io", bufs=IO_BUFS))
        outp = ctx.enter_context(tc.tile_pool(name="outp", bufs=4))
        psum = ctx.enter_context(tc.tile_pool(name="psum", bufs=4, space="PSUM"))

        # consts ride the ACT HWDGE ring so the SP ring can start streaming
        # the first x group immediately; first matmul waits on whichever
        # finishes later (~2.8us instead of ~4.9us serialized)
        wt_sb = const.tile([P, KC, OUT_F], f16)
        nc.scalar.dma_start(out=wt_sb, in_=wt_d)
        wlo_sb = const.tile([P, KC // 2, 2, OUT_F], f8)
        nc.scalar.dma_start(out=wlo_sb, in_=wlo_d)
        thr_sb = const.tile([P, OC], f32)
        nc.scalar.dma_start(out=thr_sb, in_=thr_d)

        g0 = 0
        for g, group in enumerate(GROUPS):
            xhi_sb = io.tile([P, KC, max(GROUPS)], f16, name="xhi_sb")[
                :, :, :group
            ]
            xlo_sb = io.tile([P, KC // 2, 2, max(GROUPS)], f8, name="xlo_sb")[
                :, :, :, :group
            ]
            nc.sync.dma_start(out=xhi_sb, in_=xhi_d[:, g])
            nc.sync.dma_start(out=xlo_sb, in_=xlo_d[:, g])
            for bt in range(group // BT):
                b0 = bt * BT
                for oc in range(OC):
                    ps = psum.tile([P, BT], f32, name="ps")
                    # all-hi then all-lo: the first matmuls of the kernel
                    # only need the hi half of the first group in SBUF
                    for k in range(KC):
                        nc.tensor.matmul(
                            ps,
                            wt_sb[:, k, oc * P : (oc + 1) * P],
                            xhi_sb[:, k, b0 : b0 + BT],
                            start=(k == 0),
                            stop=False,
                        )
                    # lo pass: fp8 e4m3 DoubleRow, contracts 256 per matmul
                    for c in range(KC // 2):
                        nc.tensor.matmul(
                            ps,
                            wlo_sb[:, c, :, oc * P : (oc + 1) * P],
                            xlo_sb[:, c, :, b0 : b0 + BT],
                            start=False,
                            stop=(c == KC // 2 - 1),
                            perf_mode=mybir.MatmulPerfMode.DoubleRow,
                        )
                    ob = outp.tile([P, BT], bf16, name="ob")
                    nc.vector.tensor_scalar(
                        ob,
                        ps,
                        thr_sb[:, oc : oc + 1],
                        None,
                        mybir.AluOpType.is_ge,
                    )
                    # out-DMAs ride the ACT HWDGE ring so they never block
                    # the input-DMA FIFO on the SP ring
                    nc.scalar.dma_start(
                        out=out_d[oc, :, g0 + b0 : g0 + b0 + BT], in_=ob
                    )
            g0 += group

    nc.compile()
    _CACHE["nc"] = nc
    return nc


def _prep_inputs(x, weight, bias, sign):
    """Host-side prep: fold sign into weights, build thresholds, split x into
    fp16 hi/lo, transpose to [feature, batch] per-core tiles."""
    x = np.asarray(x, dtype=np.float32)
    weight = np.asarray(weight, dtype=np.float32)
    bias = np.asarray(bias, dtype=np.float32)
    sign = np.asarray(sign, dtype=np.float32).reshape(1, OUT_F)

    wp = sign.T * weight                      # [OUT_F, IN_F], ternary
    thr = (-sign[0] * bias - np.float32(0.5)).astype(np.float32)  # [OUT_F]

    wt = np.ascontiguousarray(
        wp.T.reshape(KC, P, OUT_F).transpose(1, 0, 2)
    ).astype(np.float16)                      # [P, KC, OUT_F]
    thr2 = np.ascontiguousarray(thr.reshape(OC, P).T)  # [P, OC]

    xhi = x.astype(np.float16)
    f8np = ml_dtypes.float8_e4m3fn
    xlo8 = ((x - xhi.astype(np.float32)) * np.float32(64.0)).astype(f8np)
    wlo8 = np.ascontiguousarray(
        (wp.T * np.float32(1.0 / 64.0))
        .reshape(KC // 2, 2, P, OUT_F)
        .transpose(2, 0, 1, 3)
    ).astype(f8np)                            # [P, KC//2, 2, OUT_F]

    n_groups = len(GROUPS)
    grp = GROUPS[0]
    in_maps = []
    for c in range(N_CORES):
        sl = slice(c * B_CORE, (c + 1) * B_CORE)
        hi = np.ascontiguousarray(
            xhi[sl].reshape(n_groups, grp, KC, P).transpose(3, 0, 2, 1)
        )                                      # [P, n_groups, KC, grp]
        lo = np.ascontiguousarray(
            xlo8[sl]
            .reshape(n_groups, grp, KC // 2, 2, P)
            .transpose(4, 0, 2, 3, 1)
        )                                      # [P, n_groups, KC//2, 2, grp]
        in_maps.append(
            {"xhi": hi, "xlo8": lo, "wt": wt, "wlo8": wlo8, "thr": thr2}
        )
    return in_maps


def _assemble(results):
    """[core][OC, P, B_CORE] bf16 -> [BATCH, OUT_F] fp32"""
    full = np.concatenate(
        [r["out"].reshape(OUT_F, B_CORE) for r in results], axis=1
    )  # [OUT_F, BATCH]
    return np.ascontiguousarray(full.T).astype(np.float32)


def run(x, weight, bias, sign, trace=False):
    """Run the kernel; returns (output, BassKernelResults)."""
    from concourse.bass_utils import run_bass_kernel_spmd

    if not trace:
        # The NTFF profile hook module may be absent in this image; make
        # sure a stray BASS_TRACE=1 can't route us into the trace path.
        os.environ["BASS_NEVER_TRACE"] = "1"
    else:
        os.environ.pop("BASS_NEVER_TRACE", None)

    nc = _build()
    in_maps = _prep_inputs(x, weight, bias, sign)
    res = run_bass_kernel_spmd(
        nc,
        in_maps,
        core_ids=list(range(N_CORES)),
        trace=trace,
    )
    return _assemble(res.results), res


def kernel(x, weight, bias, sign):
    out, _ = run(x, weight, bias, sign, trace=False)
    return out
